# revision 15
# baseline (speedup 1.0000x reference)
"""Trainium2 Bass kernel for nn_DON_cnn_79216376807825 (histogram_binning).

Architecture (8 NeuronCores, data-parallel over the 262144 points):
  The reference needs max-over-points of two 4-layer tanh MLPs
  (3->256->256->256->256).  The device runs a fast fp8 screening pass and
  the host exactly rescores the tiny near-max candidate set, so the final
  params are fp32-exact while the device does ~99.5% of the FLOPs.

  - Layer 0 (0.4% of FLOPs) runs on host in fp32; h0 is quantized to
    e4m3 and streamed in per tile ([128, 2, T] per MLP: hidden dim as
    (slot, partition), points on the free axis).
  - Layers 1-3 are fp8e4 DoubleRow matmuls: contraction 256 = 2 k-slots
    of 128 in ONE 512-col pass (216 ns) -- 2x over fp16.  Weights are
    pre-scaled by a power of two so e4m3 stays in normal range;
    LDWEIGHTS hides behind the matmul stream.
  - tanh: all 8 [128,1024] blocks/tile on the ACT engine (per-partition
    bias AP + 1/alpha scale imm, fp8 out, ~1.13us each) -- ACT is the
    pacing engine.  Custom DVE ops are unusable in this runtime (no
    dve-table delivery; they crash the exec unit), and a stock-op DVE
    polynomial costs ~4x ACT per element, so DVE only handles the dump.
  - DVE also consumes the z3 PSUM: 4 casts [128,1024]->fp16 per tile
    streaming to DRAM (gpsimd SWDGE queue) as the per-point screening
    dump.
  - Software pipeline: per cycle t the PE emits z2(t), then interleaves
    z3(t) with z1(t+1); PSUM is 4 j-granular [128,1024] tiles (8 banks)
    so the ACT engine never hits a tile-boundary bubble.
  - Host screening: per-dim approx maxima from the dump; margin
    calibrated against an exact fp32 subsample; candidate points
    (typically a few thousand) rescored exactly in fp32.  The empty
    patch-995 part stays on host as before.
"""

import os
import sys

if "/opt/trn_rl_repo" not in sys.path:
    sys.path.insert(0, "/opt/trn_rl_repo")

import numpy as np

import concourse.bass as bass  # noqa: F401
import concourse.mybir as mybir
from concourse import bacc, tile
from concourse.bass_utils import run_bass_kernel_spmd

import ml_dtypes

N_CORES = 8
N_PTS = 262144
P = N_PTS // N_CORES          # 32768 points per core
T = 1024                      # points per tile
NT = P // T                   # 32 tiles
H = 256
MNK = 10
PATCH_ID = 995

F32 = mybir.dt.float32
F16 = mybir.dt.float16
F8 = mybir.dt.float8e4
AF = mybir.ActivationFunctionType
OP = mybir.AluOpType
DR = mybir.MatmulPerfMode.DoubleRow
E4 = ml_dtypes.float8_e4m3

DVE_POLY = os.environ.get("K_POLY", "1") == "1"   # (l=1, m=1, j=1) partial
PX = int(os.environ.get("K_PX", "384"))          # columns of that block on DVE

_CACHE: dict = {}

# aux column layout ([128, NAUX] f32):
#  0..7  : ACT tanh bias, col = (l-1)*4 + m*2 + j  (l in {1,2})
#  8     : alpha1*b1 for the DVE poly block (m=1, dims 128..255)
NAUX = 12


def _build(inv_scales, poly):
    """inv_scales: dict (l, m) -> ACT scale imm (1/alpha_l).
    poly: None or (nrm, k2, k1, k0) imms for the DVE block (l1, m1, j1)."""
    nc = bacc.Bacc("TRN2", target_bir_lowering=False, debug=False,
                   num_devices=N_CORES)
    h0_d = [nc.dram_tensor(f"h0{m}", [128, 2, P], F8,
                           kind="ExternalInput").ap() for m in (0, 1)]
    wk8_d = nc.dram_tensor("wk8", [128, 12, 2, 128], F8,
                           kind="ExternalInput").ap()
    aux_d = nc.dram_tensor("aux", [128, NAUX], F32, kind="ExternalInput").ap()
    zd_d = nc.dram_tensor("zd", [128, 4, P], F16, kind="ExternalOutput").ap()

    MJ = [(0, 0), (0, 1), (1, 0), (1, 1)]

    with tile.TileContext(nc) as tc:
        with tc.tile_pool(name="const", bufs=1) as cpool, \
             tc.tile_pool(name="h0p", bufs=6) as hpool, \
             tc.tile_pool(name="act", bufs=6) as apool, \
             tc.tile_pool(name="dmp", bufs=8) as dpool, \
             tc.tile_pool(name="pp", bufs=8) as ppool, \
             tc.tile_pool(name="ps", bufs=4, space="PSUM") as pspool:
            wk8_s = cpool.tile([128, 12, 2, 128], F8, tag="wk8")
            aux_s = cpool.tile([128, NAUX], F32, tag="aux")
            # warm the ACT tanh table while the input DMAs stream (in-place
            # on a scratch tile, result discarded)
            scr = cpool.tile([128, 1], F16, tag="scr")
            nc.scalar.activation(scr[:], scr[:], AF.Tanh,
                                 bias=0.0, scale=1.0)
            # split the init load: layer-1 weights + aux on sync, first h0
            # tile on gpsimd, bulk weights behind it
            nc.sync.dma_start(aux_s[:], aux_d[:])
            nc.sync.dma_start(wk8_s[:, 0:4, :, :], wk8_d[:, 0:4, :, :])

            h0t = {}     # (m, t) -> tile, alive until z1 consumed
            h1t = {}     # (m,) current/next
            h2t = {}

            def dma_h0(t, eng=None):
                for m in (0, 1):
                    ht = hpool.tile([128, 2, T], F8, tag="h0",
                                    name=f"h0_{t}_{m}")
                    (eng or nc.sync).dma_start(
                        ht[:], h0_d[m][:, :, t * T:(t + 1) * T])
                    h0t[(m, t)] = ht

            def consume_tanh(ps, l, m, j, dest):
                """psum [128,1024] -> dest[:, j, :] fp8 via ACT tanh, or the
                DVE poly for the designated block."""
                if poly is not None and (l, m, j) == (1, 1, 1):
                    # split this block: ACT takes the first T-PX columns,
                    # DVE evaluates the odd quintic on the last PX (paying
                    # its per-pass fixed costs only once per tile)
                    nrm, k2, k1, k0 = poly
                    XS = T - PX
                    c = (l - 1) * 4 + m * 2 + j
                    nc.scalar.activation(dest[:, j, 0:XS], ps[:, 0:XS],
                                         AF.Tanh, bias=aux_s[:, c:c + 1],
                                         scale=float(inv_scales[(l, m)]))
                    t16 = ppool.tile([128, PX], F16, tag="pp", name="pt")
                    s16 = ppool.tile([128, PX], F16, tag="pp", name="pss")
                    q16 = ppool.tile([128, PX], F16, tag="pp", name="pq")
                    r16 = ppool.tile([128, PX], F16, tag="pp", name="pr")
                    nc.vector.tensor_scalar(t16[:], ps[:, XS:T],
                                            aux_s[:, 8:9],
                                            float(nrm), OP.add, OP.mult)
                    nc.vector.tensor_mul(s16[:], t16[:], t16[:])
                    nc.vector.tensor_scalar(q16[:], s16[:], float(k2),
                                            float(k1), OP.mult, OP.add)
                    nc.vector.tensor_mul(r16[:], q16[:], s16[:])
                    nc.vector.scalar_tensor_tensor(dest[:, j, XS:T], r16[:],
                                                   float(k0), t16[:],
                                                   OP.add, OP.mult)
                else:
                    c = (l - 1) * 4 + m * 2 + j
                    nc.scalar.activation(dest[:, j, :], ps[:], AF.Tanh,
                                         bias=aux_s[:, c:c + 1],
                                         scale=float(inv_scales[(l, m)]))

            def mm_pair(name, blk, src, slot_src=True):
                """Two DoubleRow matmuls (cb halves) into a fresh psum."""
                ps = pspool.tile([128, T], F32, tag="ps", name=name)
                for cb in (0, 1):
                    nc.tensor.matmul(ps[:, cb * 512:(cb + 1) * 512],
                                     wk8_s[:, blk, :, :],
                                     src[:, :, cb * 512:(cb + 1) * 512],
                                     start=True, stop=True, perf_mode=DR)
                return ps

            def emit_z1(t):
                for m in (0, 1):
                    h1t[(m, t)] = apool.tile([128, 2, T], F8, tag="h",
                                             name=f"h1_{t}_{m}")
                for m, j in MJ:
                    ps = mm_pair(f"ps1_{t}_{m}_{j}", m * 2 + j, h0t[(m, t)])
                    consume_tanh(ps, 1, m, j, h1t[(m, t)])

            # prologue: first h0 tile races the sync-queue weight load on
            # the gpsimd queue; bulk weights follow it
            dma_h0(0, eng=nc.gpsimd)
            nc.gpsimd.dma_start(wk8_s[:, 4:8, :, :], wk8_d[:, 4:8, :, :])
            nc.gpsimd.dma_start(wk8_s[:, 8:12, :, :], wk8_d[:, 8:12, :, :])
            dma_h0(1)
            emit_z1(0)

            for t in range(NT):
                # phase A: z2(t) -> h2(t), ACT blocks 1-4 of the cycle
                for m in (0, 1):
                    h2t[(m, t)] = apool.tile([128, 2, T], F8, tag="h",
                                             name=f"h2_{t}_{m}")
                for m, j in MJ:
                    ps = mm_pair(f"ps2_{t}_{m}_{j}", 4 + m * 2 + j,
                                 h1t[(m, t)])
                    consume_tanh(ps, 2, m, j, h2t[(m, t)])
                h1t.pop((0, t)), h1t.pop((1, t))
                if t + 2 < NT:
                    dma_h0(t + 2)
                # phase B: interleave z3(t) with z1(t+1)
                if t + 1 < NT:
                    for m in (0, 1):
                        h1t[(m, t + 1)] = apool.tile([128, 2, T], F8, tag="h",
                                                     name=f"h1_{t + 1}_{m}")
                for m, j in MJ:
                    ps = mm_pair(f"ps3_{t}_{m}_{j}", 8 + m * 2 + j,
                                 h2t[(m, t)])
                    dp = dpool.tile([128, T], F16, tag="zd",
                                    name=f"zd_{t}_{m}_{j}")
                    if t == NT - 1:
                        # final tile: ACT has no z1(t+1) blocks left, so it
                        # absorbs the dump copies -- collapses the DVE tail;
                        # dumps split across both DMA queues to drain fast
                        nc.scalar.activation(dp[:], ps[:], AF.Copy,
                                             bias=0.0, scale=1.0)
                        eng = nc.sync if j == 0 else nc.gpsimd
                        eng.dma_start(
                            zd_d[:, 2 * m + j, t * T:(t + 1) * T], dp[:])
                    else:
                        nc.vector.tensor_copy(dp[:], ps[:])
                        nc.gpsimd.dma_start(
                            zd_d[:, 2 * m + j, t * T:(t + 1) * T], dp[:])
                    if t + 1 < NT:
                        ps1 = mm_pair(f"ps1_{t + 1}_{m}_{j}", m * 2 + j,
                                      h0t[(m, t + 1)])
                        consume_tanh(ps1, 1, m, j, h1t[(m, t + 1)])
                h2t.pop((0, t)), h2t.pop((1, t))
                h0t.pop((0, t)), h0t.pop((1, t))
    nc.compile()
    return nc


def _get_nc(inv_scales, poly):
    key = ("nc4",
           tuple(sorted((k, float(v)) for k, v in inv_scales.items())),
           tuple(round(float(v), 8) for v in poly) if poly else None)
    if key not in _CACHE:
        _CACHE[key] = _build(inv_scales, poly)
    return _CACHE[key]


def _pow2_scale(w):
    """Power-of-two alpha with max|w*alpha| ~ 12."""
    mx = float(np.abs(w).max())
    return 2.0 ** int(np.floor(np.log2(12.0 / mx)))


def _fit_odd5(Z, lim=1.05, n=1501, iters=40):
    """LSQ-minimax fit of k0 t + k1 t^3 + k2 t^5 ~ tanh(Z t), t in [0,lim]."""
    t = np.linspace(0, lim, n)
    y = np.tanh(Z * t)
    A = np.stack([t, t ** 3, t ** 5], axis=1)
    w = np.ones(n)
    k = None
    for _ in range(iters):
        k, *_ = np.linalg.lstsq(A * w[:, None], y * w, rcond=None)
        r = np.abs(A @ k - y)
        w = w * (r / (r.max() + 1e-15) + 0.2)
        w /= w.mean()
    return k  # k0, k1, k2


def _prep(x, g):
    """Host prep: h0 (fp32->e4m3), scaled fp8 DR weight blocks, aux,
    poly imms."""
    alphas = {}
    wk8 = np.zeros((128, 12, 2, 128), E4)
    aux = np.zeros((128, NAUX), np.float32)
    h0 = {}
    sub = x[::64]
    for m, pre in enumerate(("tb", "br")):
        z0 = x @ g[f"{pre}_w0"] + g[f"{pre}_b0"]
        h = np.tanh(z0)                                     # (N, 256)
        h0[m] = np.ascontiguousarray(
            h.T.reshape(2, 128, N_PTS).transpose(1, 0, 2)).astype(E4)
        for l in (1, 2, 3):
            W = g[f"{pre}_w{l}"]
            a = _pow2_scale(W)
            alphas[(l, m)] = a
            for j in (0, 1):
                blk = (l - 1) * 4 + m * 2 + j
                for slot in (0, 1):
                    wk8[:, blk, slot, :] = (
                        W[slot * 128:(slot + 1) * 128,
                          j * 128:(j + 1) * 128] * a).astype(E4)
            if l < 3:
                bvec = g[f"{pre}_b{l}"]
                for j in (0, 1):
                    aux[:, (l - 1) * 4 + m * 2 + j] = \
                        bvec[j * 128:(j + 1) * 128]
    inv_scales = {(l, m): 1.0 / alphas[(l, m)]
                  for l in (1, 2) for m in (0, 1)}

    poly = None
    if DVE_POLY:
        h0s = np.tanh(sub @ g["br_w0"] + g["br_b0"])
        z1s = h0s @ g["br_w1"] + g["br_b1"]
        Z1 = 1.3 * float(np.abs(z1s).max()) + 0.25
        a1 = alphas[(1, 1)]
        aux[:, 8] = a1 * g["br_b1"][128:256]
        k0, k1, k2 = _fit_odd5(Z1)
        poly = (1.0 / (a1 * Z1), float(k2), float(k1), float(k0))
    return h0, wk8, aux, alphas, inv_scales, poly


def _mlp_np(h, layers):
    for w, b in layers[:-1]:
        h = np.tanh(h @ w + b)
    w, b = layers[-1]
    return h @ w + b


def _exact_z3(x_pts, g, pre):
    """Exact fp32 pre-bias last-layer outputs for given points."""
    h = x_pts
    for l in range(3):
        h = np.tanh(h @ g[f"{pre}_w{l}"] + g[f"{pre}_b{l}"])
    return h @ g[f"{pre}_w3"]          # (n, 256), no b3


def _run_device(x, g, trace=False):
    """fp8 screening pass on 8 cores + host exact rescore.
    Returns (tb_pre, br_pre, res): exact pre-bias maxima (256,) each."""
    x = np.asarray(x, np.float32)
    h0, wk8, aux, alphas, inv_scales, poly = _prep(x, g)
    nc = _get_nc(inv_scales, poly)
    in_maps = []
    for c in range(N_CORES):
        in_maps.append({
            "h00": np.ascontiguousarray(h0[0][:, :, c * P:(c + 1) * P]),
            "h01": np.ascontiguousarray(h0[1][:, :, c * P:(c + 1) * P]),
            "wk8": wk8, "aux": aux})
    res = run_bass_kernel_spmd(nc, in_maps, list(range(N_CORES)),
                               trace=trace)

    sub_idx = np.arange(0, N_PTS, 64)                      # 4096 pts
    cand_sets = []
    diag = {}
    for m, pre in enumerate(("tb", "br")):
        a3 = alphas[(3, m)]
        blocks = np.stack([r["zd"][:, 2 * m:2 * m + 2, :]
                           for r in res.results])           # (8,128,2,P) f16
        zf = blocks.astype(np.float32) / a3                 # (8,128,2,P)
        M = zf.max(axis=(0, 3))                             # (128, 2)
        exact_sub = _exact_z3(x[sub_idx], g, pre)           # (4096, 256)
        core_i = sub_idx // P
        n_i = sub_idx % P
        approx_sub = zf[core_i, :, :, n_i]                  # (4096, 128, 2)
        approx_sub = approx_sub.transpose(0, 2, 1).reshape(len(sub_idx), 256)
        err = np.abs(exact_sub - approx_sub)
        emax = float(err.max())
        margin = 4.0 * emax + 0.01
        diag[pre] = (emax, float(np.sqrt((err**2).mean())), margin)
        if not np.isfinite(emax) or emax > 0.5:
            cand_sets.append(None)                          # full rescore
            continue
        thr = M - margin                                    # (128, 2)
        mask = zf >= thr[None, :, :, None]
        if int(mask.sum()) > 2_000_000:
            cand_sets.append(None)
            continue
        cc, _, _, nn = np.nonzero(mask)
        cand_sets.append(np.unique(cc * P + nn))
    if any(c is None for c in cand_sets):
        cands = np.arange(N_PTS)                            # degenerate dump
    else:
        cands = np.unique(np.concatenate(cand_sets + [sub_idx]))
    _CACHE["screen_diag"] = (diag, len(cands))

    out = []
    for m, pre in enumerate(("tb", "br")):
        best = np.full(256, -np.inf, np.float32)
        for s in range(0, len(cands), 65536):
            ze = _exact_z3(x[cands[s:s + 65536]], g, pre)
            best = np.maximum(best, ze.max(axis=0))
        out.append(best)
    return out[0], out[1], res


def kernel(x, y,
           tb_w0, tb_b0, tb_w1, tb_b1, tb_w2, tb_b2, tb_w3, tb_b3,
           br_w0, br_b0, br_w1, br_b1, br_w2, br_b2, br_w3, br_b3,
           tr_w0, tr_b0, tr_w1, tr_b1, tr_w2, tr_b2, tr_w3, tr_b3,
           o_w0, o_b0, o_w1, o_b1, o_w2, o_b2, _trace=False):
    x = np.asarray(x, np.float32)
    y = np.asarray(y, np.float32)
    g = {k: np.asarray(v, np.float32) for k, v in dict(
        tb_w0=tb_w0, tb_w1=tb_w1, tb_w2=tb_w2, tb_w3=tb_w3,
        br_w0=br_w0, br_w1=br_w1, br_w2=br_w2, br_w3=br_w3,
        tb_b0=tb_b0, tb_b1=tb_b1, tb_b2=tb_b2,
        br_b0=br_b0, br_b1=br_b1, br_b2=br_b2,
    ).items()}

    tb_pre, br_pre, res = _run_device(x, g, trace=_trace)
    _CACHE["last_results"] = res
    global_param = tb_pre + np.asarray(tb_b3, np.float32)   # (256,)
    local_param = br_pre + np.asarray(br_b3, np.float32)

    # patch gather (host): points whose bin id == PATCH_ID
    c = np.clip(np.floor(x * float(MNK)).astype(np.int64), 0, MNK - 1)
    pid = c[:, 0] * (MNK * MNK) + c[:, 1] * MNK + c[:, 2]
    idx = np.nonzero(pid == PATCH_ID)[0]
    x_patch = x[idx]
    gt_patch = y[idx]

    tr = [(np.asarray(tr_w0, np.float32), np.asarray(tr_b0, np.float32)),
          (np.asarray(tr_w1, np.float32), np.asarray(tr_b1, np.float32)),
          (np.asarray(tr_w2, np.float32), np.asarray(tr_b2, np.float32)),
          (np.asarray(tr_w3, np.float32), np.asarray(tr_b3, np.float32))]
    o = [(np.asarray(o_w0, np.float32), np.asarray(o_b0, np.float32)),
         (np.asarray(o_w1, np.float32), np.asarray(o_b1, np.float32)),
         (np.asarray(o_w2, np.float32), np.asarray(o_b2, np.float32))]

    local_coord = _mlp_np(x_patch, tr)                      # (MM, 256)
    mm = local_coord.shape[0]
    feat = np.concatenate([
        local_coord,
        np.broadcast_to(local_param, (mm, local_param.shape[0])),
        np.broadcast_to(global_param, (mm, global_param.shape[0])),
    ], axis=-1).astype(np.float32)
    pred_patch = _mlp_np(feat, o).astype(np.float32)
    return pred_patch, gt_patch


# revision 17
# speedup vs baseline: 1.0560x; 1.0560x over previous
"""Trainium2 Bass kernel for nn_DON_cnn_79216376807825 (histogram_binning).

Architecture (8 NeuronCores, data-parallel over the 262144 points):
  The reference needs max-over-points of two 4-layer tanh MLPs
  (3->256->256->256->256).  The device runs a fast fp8 screening pass and
  the host exactly rescores the tiny near-max candidate set, so the final
  params are fp32-exact while the device does ~99.5% of the FLOPs.

  - Layer 0 (0.4% of FLOPs) runs on host in fp32; h0 is quantized to
    e4m3 and streamed in per tile ([128, 2, T] per MLP: hidden dim as
    (slot, partition), points on the free axis).
  - Layers 1-3 are fp8e4 DoubleRow matmuls: contraction 256 = 2 k-slots
    of 128 in ONE 512-col pass (216 ns) -- 2x over fp16.  Weights are
    pre-scaled by a power of two so e4m3 stays in normal range;
    LDWEIGHTS hides behind the matmul stream.
  - tanh: all 8 [128,1024] blocks/tile on the ACT engine (per-partition
    bias AP + 1/alpha scale imm, fp8 out, ~1.13us each) -- ACT is the
    pacing engine.  Custom DVE ops are unusable in this runtime (no
    dve-table delivery; they crash the exec unit), and a stock-op DVE
    polynomial costs ~4x ACT per element, so DVE only handles the dump.
  - DVE also consumes the z3 PSUM: 4 casts [128,1024]->fp16 per tile
    streaming to DRAM (gpsimd SWDGE queue) as the per-point screening
    dump.
  - Software pipeline: per cycle t the PE emits z2(t), then interleaves
    z3(t) with z1(t+1); PSUM is 4 j-granular [128,1024] tiles (8 banks)
    so the ACT engine never hits a tile-boundary bubble.
  - Host screening: per-dim approx maxima from the dump; margin
    calibrated against an exact fp32 subsample; candidate points
    (typically a few thousand) rescored exactly in fp32.  The empty
    patch-995 part stays on host as before.
"""

import os
import sys

if "/opt/trn_rl_repo" not in sys.path:
    sys.path.insert(0, "/opt/trn_rl_repo")

import numpy as np

import concourse.bass as bass  # noqa: F401
import concourse.mybir as mybir
from concourse import bacc, tile
from concourse.bass_utils import run_bass_kernel_spmd

import ml_dtypes

N_CORES = 8
N_PTS = 262144
P = N_PTS // N_CORES          # 32768 points per core
T = 1024                      # points per tile
NT = P // T                   # 32 tiles
H = 256
MNK = 10
PATCH_ID = 995

F32 = mybir.dt.float32
F16 = mybir.dt.float16
F8 = mybir.dt.float8e4
AF = mybir.ActivationFunctionType
OP = mybir.AluOpType
DR = mybir.MatmulPerfMode.DoubleRow
E4 = ml_dtypes.float8_e4m3

DVE_POLY = os.environ.get("K_POLY", "0") == "1"   # (l=1, m=1, j=1) on DVE

_CACHE: dict = {}

# aux column layout ([128, NAUX] f32):
#  0..7  : ACT tanh bias, col = (l-1)*4 + m*2 + j  (l in {1,2})
#  8     : alpha1*b1 for the DVE poly block (m=1, dims 128..255)
NAUX = 12


def _build(inv_scales, poly):
    """inv_scales: dict (l, m) -> ACT scale imm (1/alpha_l).
    poly: None or (nrm, k2, k1, k0) imms for the DVE block (l1, m1, j1)."""
    nc = bacc.Bacc("TRN2", target_bir_lowering=False, debug=False,
                   num_devices=N_CORES)
    h0_d = [nc.dram_tensor(f"h0{m}", [128, 2, P], F8,
                           kind="ExternalInput").ap() for m in (0, 1)]
    wk8_d = nc.dram_tensor("wk8", [128, 12, 2, 128], F8,
                           kind="ExternalInput").ap()
    aux_d = nc.dram_tensor("aux", [128, NAUX], F32, kind="ExternalInput").ap()
    zd_d = nc.dram_tensor("zd", [128, 4, P], F16, kind="ExternalOutput").ap()

    MJ = [(0, 0), (0, 1), (1, 0), (1, 1)]

    with tile.TileContext(nc) as tc:
        with tc.tile_pool(name="const", bufs=1) as cpool, \
             tc.tile_pool(name="h0p", bufs=6) as hpool, \
             tc.tile_pool(name="act", bufs=6) as apool, \
             tc.tile_pool(name="dmp", bufs=8) as dpool, \
             tc.tile_pool(name="pp", bufs=8) as ppool, \
             tc.tile_pool(name="ps", bufs=4, space="PSUM") as pspool:
            wk8_s = cpool.tile([128, 12, 2, 128], F8, tag="wk8")
            aux_s = cpool.tile([128, NAUX], F32, tag="aux")
            # warm the ACT tanh table while the input DMAs stream (in-place
            # on a scratch tile, result discarded)
            scr = cpool.tile([128, 1], F16, tag="scr")
            nc.scalar.activation(scr[:], scr[:], AF.Tanh,
                                 bias=0.0, scale=1.0)
            # split the init load: layer-1 weights + aux on sync, first h0
            # tile on gpsimd, bulk weights behind it
            nc.sync.dma_start(aux_s[:], aux_d[:])
            nc.sync.dma_start(wk8_s[:, 0:4, :, :], wk8_d[:, 0:4, :, :])

            h0t = {}     # (m, t) -> tile, alive until z1 consumed
            h1t = {}     # (m,) current/next
            h2t = {}

            def dma_h0(t, eng=None):
                for m in (0, 1):
                    ht = hpool.tile([128, 2, T], F8, tag="h0",
                                    name=f"h0_{t}_{m}")
                    (eng or nc.sync).dma_start(
                        ht[:], h0_d[m][:, :, t * T:(t + 1) * T])
                    h0t[(m, t)] = ht

            def consume_tanh(ps, l, m, j, dest):
                """psum [128,1024] -> dest[:, j, :] fp8 via ACT tanh, or the
                DVE poly for the designated block."""
                if poly is not None and (l, m, j) == (1, 1, 1):
                    nrm, k2, k1, k0 = poly
                    t16 = ppool.tile([128, T], F16, tag="pp", name="pt")
                    s16 = ppool.tile([128, T], F16, tag="pp", name="pss")
                    q16 = ppool.tile([128, T], F16, tag="pp", name="pq")
                    r16 = ppool.tile([128, T], F16, tag="pp", name="pr")
                    nc.vector.tensor_scalar(t16[:], ps[:], aux_s[:, 8:9],
                                            float(nrm), OP.add, OP.mult)
                    nc.vector.tensor_mul(s16[:], t16[:], t16[:])
                    nc.vector.tensor_scalar(q16[:], s16[:], float(k2),
                                            float(k1), OP.mult, OP.add)
                    nc.vector.tensor_mul(r16[:], q16[:], s16[:])
                    nc.vector.scalar_tensor_tensor(dest[:, j, :], r16[:],
                                                   float(k0), t16[:],
                                                   OP.add, OP.mult)
                else:
                    c = (l - 1) * 4 + m * 2 + j
                    nc.scalar.activation(dest[:, j, :], ps[:], AF.Tanh,
                                         bias=aux_s[:, c:c + 1],
                                         scale=float(inv_scales[(l, m)]))

            def mm_pair(name, blk, src, slot_src=True):
                """Two DoubleRow matmuls (cb halves) into a fresh psum."""
                ps = pspool.tile([128, T], F32, tag="ps", name=name)
                for cb in (0, 1):
                    nc.tensor.matmul(ps[:, cb * 512:(cb + 1) * 512],
                                     wk8_s[:, blk, :, :],
                                     src[:, :, cb * 512:(cb + 1) * 512],
                                     start=True, stop=True, perf_mode=DR)
                return ps

            def emit_z1(t):
                for m in (0, 1):
                    h1t[(m, t)] = apool.tile([128, 2, T], F8, tag="h",
                                             name=f"h1_{t}_{m}")
                for m, j in MJ:
                    ps = mm_pair(f"ps1_{t}_{m}_{j}", m * 2 + j, h0t[(m, t)])
                    consume_tanh(ps, 1, m, j, h1t[(m, t)])

            # prologue: first h0 tile races the sync-queue weight load on
            # the gpsimd queue; bulk weights follow it
            dma_h0(0, eng=nc.gpsimd)
            nc.gpsimd.dma_start(wk8_s[:, 4:8, :, :], wk8_d[:, 4:8, :, :])
            nc.gpsimd.dma_start(wk8_s[:, 8:12, :, :], wk8_d[:, 8:12, :, :])
            dma_h0(1)
            emit_z1(0)

            for t in range(NT):
                # phase A: z2(t) -> h2(t), ACT blocks 1-4 of the cycle
                for m in (0, 1):
                    h2t[(m, t)] = apool.tile([128, 2, T], F8, tag="h",
                                             name=f"h2_{t}_{m}")
                for m, j in MJ:
                    ps = mm_pair(f"ps2_{t}_{m}_{j}", 4 + m * 2 + j,
                                 h1t[(m, t)])
                    consume_tanh(ps, 2, m, j, h2t[(m, t)])
                h1t.pop((0, t)), h1t.pop((1, t))
                if t + 2 < NT:
                    dma_h0(t + 2)
                # phase B: interleave z3(t) with z1(t+1)
                if t + 1 < NT:
                    for m in (0, 1):
                        h1t[(m, t + 1)] = apool.tile([128, 2, T], F8, tag="h",
                                                     name=f"h1_{t + 1}_{m}")
                for m, j in MJ:
                    ps = mm_pair(f"ps3_{t}_{m}_{j}", 8 + m * 2 + j,
                                 h2t[(m, t)])
                    dp = dpool.tile([128, T], F16, tag="zd",
                                    name=f"zd_{t}_{m}_{j}")
                    if t == NT - 1:
                        # final tile: ACT has no z1(t+1) blocks left, so it
                        # absorbs the dump copies -- collapses the DVE tail;
                        # dumps split across both DMA queues to drain fast
                        nc.scalar.activation(dp[:], ps[:], AF.Copy,
                                             bias=0.0, scale=1.0)
                        nc.sync.dma_start(
                            zd_d[:, 2 * m + j, t * T:(t + 1) * T], dp[:])
                    else:
                        nc.vector.tensor_copy(dp[:], ps[:])
                        nc.gpsimd.dma_start(
                            zd_d[:, 2 * m + j, t * T:(t + 1) * T], dp[:])
                    if t + 1 < NT:
                        ps1 = mm_pair(f"ps1_{t + 1}_{m}_{j}", m * 2 + j,
                                      h0t[(m, t + 1)])
                        consume_tanh(ps1, 1, m, j, h1t[(m, t + 1)])
                h2t.pop((0, t)), h2t.pop((1, t))
                h0t.pop((0, t)), h0t.pop((1, t))
    nc.compile()
    return nc


def _get_nc(inv_scales, poly):
    key = ("nc4",
           tuple(sorted((k, float(v)) for k, v in inv_scales.items())),
           tuple(round(float(v), 8) for v in poly) if poly else None)
    if key not in _CACHE:
        _CACHE[key] = _build(inv_scales, poly)
    return _CACHE[key]


def _pow2_scale(w):
    """Power-of-two alpha with max|w*alpha| ~ 12."""
    mx = float(np.abs(w).max())
    return 2.0 ** int(np.floor(np.log2(12.0 / mx)))


def _fit_odd5(Z, lim=1.05, n=1501, iters=40):
    """LSQ-minimax fit of k0 t + k1 t^3 + k2 t^5 ~ tanh(Z t), t in [0,lim]."""
    t = np.linspace(0, lim, n)
    y = np.tanh(Z * t)
    A = np.stack([t, t ** 3, t ** 5], axis=1)
    w = np.ones(n)
    k = None
    for _ in range(iters):
        k, *_ = np.linalg.lstsq(A * w[:, None], y * w, rcond=None)
        r = np.abs(A @ k - y)
        w = w * (r / (r.max() + 1e-15) + 0.2)
        w /= w.mean()
    return k  # k0, k1, k2


def _prep(x, g):
    """Host prep: h0 (fp32->e4m3), scaled fp8 DR weight blocks, aux,
    poly imms."""
    alphas = {}
    wk8 = np.zeros((128, 12, 2, 128), E4)
    aux = np.zeros((128, NAUX), np.float32)
    h0 = {}
    sub = x[::64]
    for m, pre in enumerate(("tb", "br")):
        z0 = x @ g[f"{pre}_w0"] + g[f"{pre}_b0"]
        h = np.tanh(z0)                                     # (N, 256)
        h0[m] = np.ascontiguousarray(
            h.T.reshape(2, 128, N_PTS).transpose(1, 0, 2)).astype(E4)
        for l in (1, 2, 3):
            W = g[f"{pre}_w{l}"]
            a = _pow2_scale(W)
            alphas[(l, m)] = a
            for j in (0, 1):
                blk = (l - 1) * 4 + m * 2 + j
                for slot in (0, 1):
                    wk8[:, blk, slot, :] = (
                        W[slot * 128:(slot + 1) * 128,
                          j * 128:(j + 1) * 128] * a).astype(E4)
            if l < 3:
                bvec = g[f"{pre}_b{l}"]
                for j in (0, 1):
                    aux[:, (l - 1) * 4 + m * 2 + j] = \
                        bvec[j * 128:(j + 1) * 128]
    inv_scales = {(l, m): 1.0 / alphas[(l, m)]
                  for l in (1, 2) for m in (0, 1)}

    poly = None
    if DVE_POLY:
        h0s = np.tanh(sub @ g["br_w0"] + g["br_b0"])
        z1s = h0s @ g["br_w1"] + g["br_b1"]
        Z1 = 1.3 * float(np.abs(z1s).max()) + 0.25
        a1 = alphas[(1, 1)]
        aux[:, 8] = a1 * g["br_b1"][128:256]
        k0, k1, k2 = _fit_odd5(Z1)
        poly = (1.0 / (a1 * Z1), float(k2), float(k1), float(k0))
    return h0, wk8, aux, alphas, inv_scales, poly


def _mlp_np(h, layers):
    for w, b in layers[:-1]:
        h = np.tanh(h @ w + b)
    w, b = layers[-1]
    return h @ w + b


def _exact_z3(x_pts, g, pre):
    """Exact fp32 pre-bias last-layer outputs for given points."""
    h = x_pts
    for l in range(3):
        h = np.tanh(h @ g[f"{pre}_w{l}"] + g[f"{pre}_b{l}"])
    return h @ g[f"{pre}_w3"]          # (n, 256), no b3


def _run_device(x, g, trace=False):
    """fp8 screening pass on 8 cores + host exact rescore.
    Returns (tb_pre, br_pre, res): exact pre-bias maxima (256,) each."""
    x = np.asarray(x, np.float32)
    h0, wk8, aux, alphas, inv_scales, poly = _prep(x, g)
    nc = _get_nc(inv_scales, poly)
    in_maps = []
    for c in range(N_CORES):
        in_maps.append({
            "h00": np.ascontiguousarray(h0[0][:, :, c * P:(c + 1) * P]),
            "h01": np.ascontiguousarray(h0[1][:, :, c * P:(c + 1) * P]),
            "wk8": wk8, "aux": aux})
    res = run_bass_kernel_spmd(nc, in_maps, list(range(N_CORES)),
                               trace=trace)

    sub_idx = np.arange(0, N_PTS, 64)                      # 4096 pts
    cand_sets = []
    diag = {}
    for m, pre in enumerate(("tb", "br")):
        a3 = alphas[(3, m)]
        blocks = np.stack([r["zd"][:, 2 * m:2 * m + 2, :]
                           for r in res.results])           # (8,128,2,P) f16
        zf = blocks.astype(np.float32) / a3                 # (8,128,2,P)
        M = zf.max(axis=(0, 3))                             # (128, 2)
        exact_sub = _exact_z3(x[sub_idx], g, pre)           # (4096, 256)
        core_i = sub_idx // P
        n_i = sub_idx % P
        approx_sub = zf[core_i, :, :, n_i]                  # (4096, 128, 2)
        approx_sub = approx_sub.transpose(0, 2, 1).reshape(len(sub_idx), 256)
        err = np.abs(exact_sub - approx_sub)
        emax = float(err.max())
        margin = 4.0 * emax + 0.01
        diag[pre] = (emax, float(np.sqrt((err**2).mean())), margin)
        if not np.isfinite(emax) or emax > 0.5:
            cand_sets.append(None)                          # full rescore
            continue
        thr = M - margin                                    # (128, 2)
        mask = zf >= thr[None, :, :, None]
        if int(mask.sum()) > 2_000_000:
            cand_sets.append(None)
            continue
        cc, _, _, nn = np.nonzero(mask)
        cand_sets.append(np.unique(cc * P + nn))
    if any(c is None for c in cand_sets):
        cands = np.arange(N_PTS)                            # degenerate dump
    else:
        cands = np.unique(np.concatenate(cand_sets + [sub_idx]))
    _CACHE["screen_diag"] = (diag, len(cands))

    out = []
    for m, pre in enumerate(("tb", "br")):
        best = np.full(256, -np.inf, np.float32)
        for s in range(0, len(cands), 65536):
            ze = _exact_z3(x[cands[s:s + 65536]], g, pre)
            best = np.maximum(best, ze.max(axis=0))
        out.append(best)
    return out[0], out[1], res


def kernel(x, y,
           tb_w0, tb_b0, tb_w1, tb_b1, tb_w2, tb_b2, tb_w3, tb_b3,
           br_w0, br_b0, br_w1, br_b1, br_w2, br_b2, br_w3, br_b3,
           tr_w0, tr_b0, tr_w1, tr_b1, tr_w2, tr_b2, tr_w3, tr_b3,
           o_w0, o_b0, o_w1, o_b1, o_w2, o_b2, _trace=False):
    x = np.asarray(x, np.float32)
    y = np.asarray(y, np.float32)
    g = {k: np.asarray(v, np.float32) for k, v in dict(
        tb_w0=tb_w0, tb_w1=tb_w1, tb_w2=tb_w2, tb_w3=tb_w3,
        br_w0=br_w0, br_w1=br_w1, br_w2=br_w2, br_w3=br_w3,
        tb_b0=tb_b0, tb_b1=tb_b1, tb_b2=tb_b2,
        br_b0=br_b0, br_b1=br_b1, br_b2=br_b2,
    ).items()}

    tb_pre, br_pre, res = _run_device(x, g, trace=_trace)
    _CACHE["last_results"] = res
    global_param = tb_pre + np.asarray(tb_b3, np.float32)   # (256,)
    local_param = br_pre + np.asarray(br_b3, np.float32)

    # patch gather (host): points whose bin id == PATCH_ID
    c = np.clip(np.floor(x * float(MNK)).astype(np.int64), 0, MNK - 1)
    pid = c[:, 0] * (MNK * MNK) + c[:, 1] * MNK + c[:, 2]
    idx = np.nonzero(pid == PATCH_ID)[0]
    x_patch = x[idx]
    gt_patch = y[idx]

    tr = [(np.asarray(tr_w0, np.float32), np.asarray(tr_b0, np.float32)),
          (np.asarray(tr_w1, np.float32), np.asarray(tr_b1, np.float32)),
          (np.asarray(tr_w2, np.float32), np.asarray(tr_b2, np.float32)),
          (np.asarray(tr_w3, np.float32), np.asarray(tr_b3, np.float32))]
    o = [(np.asarray(o_w0, np.float32), np.asarray(o_b0, np.float32)),
         (np.asarray(o_w1, np.float32), np.asarray(o_b1, np.float32)),
         (np.asarray(o_w2, np.float32), np.asarray(o_b2, np.float32))]

    local_coord = _mlp_np(x_patch, tr)                      # (MM, 256)
    mm = local_coord.shape[0]
    feat = np.concatenate([
        local_coord,
        np.broadcast_to(local_param, (mm, local_param.shape[0])),
        np.broadcast_to(global_param, (mm, global_param.shape[0])),
    ], axis=-1).astype(np.float32)
    pred_patch = _mlp_np(feat, o).astype(np.float32)
    return pred_patch, gt_patch
